# revision 18
# baseline (speedup 1.0000x reference)
"""Trainium2 Bass kernel for nn_ConvblockNofrills (dense_cnn).

Reference computation (per batch b, output position l, channel d):
    gate[b,l,d,k] = tanh( sum_c x[b, l+K-1, c] * weights[d, c, k] )
    out[b,l,d]    = sum_k x[b, l+k, d] * gate[b,l,d,k]
with B=8, T=4096, C=D=512, K=7, L=T-K+1=4090.

Strategy: data-parallel across the 8 NeuronCores (one batch each).
Per core everything runs in transposed (channel, position) layout:
  - gates via bf16 matmul on TensorE (fp32 PSUM accumulation)
  - tanh on ScalarE, output bf16 to SBUF
  - 7-tap multiply/accumulate on VectorE in bf16

The 896 matmuls (7 taps x 4 d-chunks x 4 c-chunks x 8 l-tiles of 512)
stream back-to-back at the warm-PE floor (~216 ns each, ~193 us), so the
optimization targets are the edges:
  - HW-measured DMA model: descriptor issue is ~0.61us each (serial per
    queue; sync + scalar are the two HWDGE queues) and sustained payload
    is ~150-210 GB/s with ~128KB descriptors. Inputs go as ~128KB
    descriptors in exact consumption order: w[k0] on the scalar queue in
    parallel with x blocks on sync, so the critical ~0.6MB lands ~12us.
  - the k=0 sweep of quad 0 runs as four single-l-tile passes so real
    matmuls start as soon as x block i has landed, with a warmup matmul
    burst sized to cover the DMA window and ramp the PE clock out of its
    cold 1.2 GHz state (any >3.4us PE gap re-throttles it).
  - PSUM groups are paired into [128,2,512] tiles (2 adjacent banks) and
    tanh runs 1024 wide: half the ScalarE instructions, which keeps the
    scalar->DVE->PE semaphore chain off the matmul critical path.
  - the final tap's units pipeline tanh/mul/add per l-tile pair and the
    last-quad stores split across both DGE queues, so the post-last-
    matmul tail is short.
x is zero-padded to 4104 cols host-side (518-col overlapping blocks) so
every tile is a uniform 512 wide; garbage tail cols are trimmed on host.
"""

import numpy as np
import ml_dtypes

import sys
for _p in ("/opt/trn_rl_repo", "/root/.axon_site/_ro/trn_rl_repo"):
    if _p not in sys.path:
        sys.path.append(_p)

B, T, C, K = 8, 4096, 512, 7
L = T - K + 1     # 4090
NCORES = 8
P = 128           # partitions
DC = C // P       # 4 chunks (of both c and d)
NL = 512          # l-tile (one PSUM bank of fp32)
NLT = 8           # l-tiles (last is ragged: 506 valid cols)
QUAD = 4          # l-tiles per accumulation quad
NQ = NLT // QUAD  # 2 quads
BW = NL + K - 1   # 518: x block width (l-tile + halo)
TPAD = 4104       # padded x columns
NWARM = 14

_cache = {}


def _build():
    import concourse.bass as bass  # noqa: F401
    import concourse.mybir as mybir
    import concourse.tile as tile
    from concourse import bacc

    bf16 = mybir.dt.bfloat16
    f32 = mybir.dt.float32
    Tanh = mybir.ActivationFunctionType.Tanh

    nc = bacc.Bacc("TRN2", target_bir_lowering=False, debug=False,
                   num_devices=NCORES)

    x_d = nc.dram_tensor("xP", [P, DC, TPAD], bf16, kind="ExternalInput")
    # wQ[k, dc, p, cc, j] = weights[dc*128+j, cc*128+p, k]
    w_d = nc.dram_tensor("wP", [K, DC, P, DC, P], bf16, kind="ExternalInput")
    o_d = nc.dram_tensor("outP", [P, DC, NLT, NL], bf16,
                         kind="ExternalOutput")

    with tile.TileContext(nc) as tc:
        with (
            tc.tile_pool(name="wpool", bufs=1) as wpool,
            tc.tile_pool(name="xpool", bufs=1) as xpool,
            tc.tile_pool(name="g0pool", bufs=6) as g0pool,
            tc.tile_pool(name="gpool", bufs=10) as gpool,
            tc.tile_pool(name="apool", bufs=10) as apool,
            tc.tile_pool(name="ppool", bufs=6) as ppool,
            tc.tile_pool(name="psum", bufs=4, space="PSUM") as psum_pool,
        ):
            # xT[p, cc, blk, j] = x[blk*512 + j, cc*128+p]; blocks overlap
            # by K-1 cols so each l-tile is self-contained.
            xT = xpool.tile([P, DC, NLT, BW], bf16, name="xT")
            # w_sb[p, k, dc, cc, j]: lhsT for (cc,k,dc) = w_sb[:,k,dc,cc,:]
            w_sb = wpool.tile([P, K, DC, DC, P], bf16, name="w_sb")

            def ld_x(blk, cc):
                c0 = blk * NL
                nc.sync.dma_start(xT[:, cc, blk, :],
                                  x_d.ap()[:, cc, c0:c0 + BW])

            def ld_w(k, dc, eng):
                eng.dma_start(w_sb[:, k, dc, :, :], w_d.ap()[k, dc])

            # DMA issue order == consumption order; ~128KB descriptors
            # (both sides 1KB-contiguous runs), critical set first. w[k0]
            # rides the scalar DGE queue (idle until the first tanh) in
            # parallel with the x blocks on sync; later weights go on
            # sync between x3 and x4 so they don't steal early bandwidth
            # from the critical x block 0.
            for dc in range(DC):
                ld_w(0, dc, nc.scalar)
            for blk in range(QUAD):
                for cc in range(DC):
                    ld_x(blk, cc)
            for k in range(1, K):
                for dc in range(DC):
                    ld_w(k, dc, nc.sync)
            for blk in range(QUAD, NLT):
                for cc in range(DC):
                    ld_x(blk, cc)

            # PE clock ramps (~1.2 -> 2.4 GHz) after ~3.4us of sustained
            # activity. Dummy matmuls bridge until the critical DMA set
            # lands (~12us: preamble ~7us + issue + ~0.6MB transfer).
            warm = wpool.tile([P, NL], bf16, name="warm")
            nc.gpsimd.memset(warm[:], 1.0)
            warm_ps = psum_pool.tile([P, 2, NL], f32, tag="ps",
                                     name="warm_ps")
            for i in range(NWARM):
                nc.tensor.matmul(warm_ps[:, 0, :], warm[:, :P], warm,
                                 start=True, stop=(i == NWARM - 1))

            for lq in range(NQ):
                q0 = lq * QUAD
                acc = [None] * DC
                if lq == 0:
                    sched = [(0, 0)] + [(k, dc) for k in range(1, K)
                                        for dc in range(DC)]
                else:
                    # dc-major: each d-chunk finishes its 7-tap sweep and
                    # stores ~24us apart, hiding output DMA expansion
                    # behind compute; only the final chunk's stores
                    # remain near the end of the kernel.
                    sched = [(k, dc) for dc in range(DC) for k in range(K)]
                for k, dc in sched:
                    if lq == 0 and k == 0:
                        # Single-l-tile passes: pass i needs only x block
                        # i (+ w[k0]), so real matmuls start ~5us earlier
                        # than a quad-wide sweep needing 4 blocks. d-chunk
                        # pairs share a 2-bank PSUM tile and one tanh.
                        for dc in range(DC):
                            acc[dc] = apool.tile([P, QUAD, NL], bf16,
                                                 tag="acc", name=f"acc0_{dc}")
                        for i in range(QUAD):
                            for dp in range(DC // 2):
                                ps = psum_pool.tile([P, 2, NL], f32,
                                                    tag="ps",
                                                    name=f"ps0_{i}_{dp}")
                                for dh in range(2):
                                    dc = 2 * dp + dh
                                    for cc in range(DC):
                                        nc.tensor.matmul(
                                            ps[:, dh, :],
                                            w_sb[:, 0, dc, cc, :],
                                            xT[:, cc, i, K - 1:BW],
                                            start=(cc == 0),
                                            stop=(cc == DC - 1))
                                g0 = g0pool.tile([P, 2, NL], bf16, tag="g0",
                                                 name=f"g0_{i}_{dp}")
                                nc.scalar.activation(g0, ps, Tanh)
                                for dh in range(2):
                                    dc = 2 * dp + dh
                                    nc.vector.tensor_mul(
                                        acc[dc][:, i, :], g0[:, dh, :],
                                        xT[:, dc, i, 0:NL])
                        continue
                    last_k = (k == K - 1)
                    for dc in (dc,):
                        ps = [psum_pool.tile([P, 2, NL], f32, tag="ps",
                                             name=f"ps_{lq}_{k}_{dc}_{p2}")
                              for p2 in range(2)]
                        if last_k:
                            # pair-wise pipelined epilogue: each 2-bank
                            # PSUM pair closes 8 MMs in, so tanh/mul/add
                            # run under the remaining matmuls and the
                            # post-last-MM tail is one pair deep.
                            g = gpool.tile([P, QUAD, NL], bf16, tag="g",
                                           name=f"gL_{lq}_{dc}")
                            nxt = apool.tile([P, QUAD, NL], bf16, tag="acc",
                                             name=f"accL_{lq}_{dc}")
                            per_lt_store = (lq == NQ - 1 and dc == DC - 1)
                            # the very last unit runs single-bank tanh per
                            # l-tile — and splits the final l-tile into
                            # 384+128 cols — so the post-last-matmul chain
                            # is a 128-col tanh/mul/add/store.
                            split_tanh = (lq == NQ - 1 and dc == DC - 1)
                            if split_tanh:
                                ps2 = psum_pool.tile([P, 2, NL], f32,
                                                     tag="ps", name="psF")
                                SPL = NL - 128
                                segs = [(0, 0, NL), (1, 0, NL), (2, 0, NL),
                                        (3, 0, SPL), (3, SPL, NL)]
                                slots = [ps[0][:, 0, :], ps[0][:, 1, :],
                                         ps[1][:, 0, :], ps[1][:, 1, :],
                                         ps2[:, 0, :]]
                                for si, (i, c0, c1) in enumerate(segs):
                                    w_ = c1 - c0
                                    for cc in range(DC):
                                        nc.tensor.matmul(
                                            slots[si][:, 0:w_],
                                            w_sb[:, k, dc, cc, :],
                                            xT[:, cc, q0 + i,
                                               K - 1 + c0:K - 1 + c1],
                                            start=(cc == 0),
                                            stop=(cc == DC - 1))
                                    nc.scalar.activation(
                                        g[:, i, c0:c1], slots[si][:, 0:w_],
                                        Tanh)
                                    prod = ppool.tile([P, NL], bf16,
                                                      tag="prodL",
                                                      name=f"prodF_{si}")
                                    nc.vector.tensor_mul(
                                        prod[:, 0:w_], g[:, i, c0:c1],
                                        xT[:, dc, q0 + i, k + c0:k + c1])
                                    nc.vector.tensor_add(
                                        nxt[:, i, c0:c1],
                                        acc[dc][:, i, c0:c1], prod[:, 0:w_])
                                    nc.sync.dma_start(
                                        o_d.ap()[:, dc, q0 + i, c0:c1],
                                        nxt[:, i, c0:c1])
                                acc[dc] = None
                                continue
                            for p2 in range(2):
                                for ih in range(2):
                                    i = 2 * p2 + ih
                                    for cc in range(DC):
                                        nc.tensor.matmul(
                                            ps[p2][:, ih, :],
                                            w_sb[:, k, dc, cc, :],
                                            xT[:, cc, q0 + i, K - 1:BW],
                                            start=(cc == 0),
                                            stop=(cc == DC - 1))
                                nc.scalar.activation(
                                    g[:, 2 * p2:2 * p2 + 2, :], ps[p2],
                                    Tanh)
                                for ih in range(2):
                                    i = 2 * p2 + ih
                                    prod = ppool.tile([P, NL], bf16,
                                                      tag="prodL",
                                                      name=f"prodL_{i}")
                                    nc.vector.tensor_mul(
                                        prod, g[:, i, :],
                                        xT[:, dc, q0 + i, k:k + NL])
                                    nc.vector.tensor_add(
                                        nxt[:, i, :], acc[dc][:, i, :], prod)
                            eng = (nc.scalar
                                   if lq == NQ - 1 and dc % 2 == 0
                                   else nc.sync)
                            eng.dma_start(
                                o_d.ap()[:, dc, q0:q0 + QUAD, :],
                                nxt[:, :, :])
                            acc[dc] = None
                            continue
                        for cc in range(DC):
                            for i in range(QUAD):
                                nc.tensor.matmul(
                                    ps[i // 2][:, i % 2, :],
                                    w_sb[:, k, dc, cc, :],
                                    xT[:, cc, q0 + i, K - 1:BW],
                                    start=(cc == 0), stop=(cc == DC - 1))
                        g = gpool.tile([P, QUAD, NL], bf16, tag="g",
                                       name=f"g_{lq}_{k}_{dc}")
                        for p2 in range(2):
                            nc.scalar.activation(
                                g[:, 2 * p2:2 * p2 + 2, :], ps[p2], Tanh)
                        xu = xT[:, dc, q0:q0 + QUAD, k:k + NL]
                        if acc[dc] is None:
                            a0 = apool.tile([P, QUAD, NL], bf16, tag="acc",
                                            name=f"acc_{lq}_{k}_{dc}")
                            nc.vector.tensor_mul(a0[:, :, :], g[:, :, :], xu)
                            acc[dc] = a0
                        else:
                            prod = ppool.tile([P, QUAD, NL], bf16,
                                              tag="prod",
                                              name=f"prod_{lq}_{k}_{dc}")
                            nc.vector.tensor_mul(prod[:, :, :], g[:, :, :],
                                                 xu)
                            nxt = apool.tile([P, QUAD, NL], bf16, tag="acc",
                                             name=f"accn_{lq}_{k}_{dc}")
                            nc.vector.tensor_add(nxt[:, :, :],
                                                 acc[dc][:, :, :],
                                                 prod[:, :, :])
                            acc[dc] = nxt

    nc.compile()
    return nc


def _prep_inputs(x, weights):
    bf = ml_dtypes.bfloat16
    # wQ[k, dc, p, cc, j] = weights[dc*128+j, cc*128+p, k]
    wP = np.ascontiguousarray(
        weights.reshape(DC, P, DC, P, K).transpose(4, 0, 3, 2, 1)).astype(bf)
    in_maps = []
    for b in range(B):
        xb = np.zeros((P, DC, TPAD), dtype=bf)
        xb[:, :, :T] = x[b].astype(bf).reshape(T, DC, P).transpose(2, 1, 0)
        in_maps.append({"xP": xb, "wP": wP})
    return in_maps


def kernel(x, weights):
    x = np.asarray(x, dtype=np.float32)
    weights = np.asarray(weights, dtype=np.float32)
    assert x.shape == (B, T, C) and weights.shape == (C, C, K)

    from concourse.bass_utils import run_bass_kernel_spmd

    if "nc" not in _cache:
        _cache["nc"] = _build()
    nc = _cache["nc"]

    in_maps = _prep_inputs(x, weights)
    res = run_bass_kernel_spmd(nc, in_maps, list(range(NCORES)))

    out = np.empty((B, L, C), dtype=np.float32)
    for b in range(B):
        op = res.results[b]["outP"].astype(np.float32)  # (P, DC, NLT, NL)
        out[b] = op.transpose(2, 3, 1, 0).reshape(NLT * NL, C)[:L]
    return out


if __name__ == "__main__":
    rng = np.random.default_rng(0)
    x = rng.standard_normal((B, T, C), dtype=np.float32)
    w = (rng.standard_normal((C, C, K), dtype=np.float32)
         / np.sqrt(np.float32(C * K)))
    out = kernel(x, w)
    print("out", out.shape, out.dtype, float(np.abs(out).max()))


# revision 21
# speedup vs baseline: 1.0022x; 1.0022x over previous
"""Trainium2 Bass kernel for nn_ConvblockNofrills (dense_cnn).

Reference computation (per batch b, output position l, channel d):
    gate[b,l,d,k] = tanh( sum_c x[b, l+K-1, c] * weights[d, c, k] )
    out[b,l,d]    = sum_k x[b, l+k, d] * gate[b,l,d,k]
with B=8, T=4096, C=D=512, K=7, L=T-K+1=4090.

Strategy: data-parallel across the 8 NeuronCores (one batch each).
Per core everything runs in transposed (channel, position) layout:
  - gates via bf16 matmul on TensorE (fp32 PSUM accumulation)
  - tanh on ScalarE, output bf16 to SBUF
  - 7-tap multiply/accumulate on VectorE in bf16

The 896 matmuls (7 taps x 4 d-chunks x 4 c-chunks x 8 l-tiles of 512)
stream back-to-back at the warm-PE floor (~216 ns each, ~193 us), so the
optimization targets are the edges:
  - HW-measured DMA model: descriptor issue is ~0.61us each (serial per
    queue; sync + scalar are the two HWDGE queues) and sustained payload
    is ~150-210 GB/s with ~128KB descriptors. Inputs go as ~128KB
    descriptors in exact consumption order: w[k0] on the scalar queue in
    parallel with x blocks on sync, so the critical ~0.6MB lands ~12us.
  - the k=0 sweep of quad 0 runs as four single-l-tile passes so real
    matmuls start as soon as x block i has landed, with a warmup matmul
    burst sized to cover the DMA window and ramp the PE clock out of its
    cold 1.2 GHz state (any >3.4us PE gap re-throttles it).
  - PSUM groups are paired into [128,2,512] tiles (2 adjacent banks) and
    tanh runs 1024 wide: half the ScalarE instructions, which keeps the
    scalar->DVE->PE semaphore chain off the matmul critical path.
  - the final tap's units pipeline tanh/mul/add per l-tile pair and the
    last-quad stores split across both DGE queues, so the post-last-
    matmul tail is short.
x is zero-padded to 4104 cols host-side (518-col overlapping blocks) so
every tile is a uniform 512 wide; garbage tail cols are trimmed on host.
"""

import numpy as np
import ml_dtypes

import sys
for _p in ("/opt/trn_rl_repo", "/root/.axon_site/_ro/trn_rl_repo"):
    if _p not in sys.path:
        sys.path.append(_p)

B, T, C, K = 8, 4096, 512, 7
L = T - K + 1     # 4090
NCORES = 8
P = 128           # partitions
DC = C // P       # 4 chunks (of both c and d)
NL = 512          # l-tile (one PSUM bank of fp32)
NLT = 8           # l-tiles (last is ragged: 506 valid cols)
QUAD = 4          # l-tiles per accumulation quad
NQ = NLT // QUAD  # 2 quads
BW = NL + K - 1   # 518: x block width (l-tile + halo)
TPAD = 4104       # padded x columns
NWARM = 14

_cache = {}


def _build():
    import concourse.bass as bass  # noqa: F401
    import concourse.mybir as mybir
    import concourse.tile as tile
    from concourse import bacc

    bf16 = mybir.dt.bfloat16
    f32 = mybir.dt.float32
    Tanh = mybir.ActivationFunctionType.Tanh

    nc = bacc.Bacc("TRN2", target_bir_lowering=False, debug=False,
                   num_devices=NCORES)

    x_d = nc.dram_tensor("xP", [P, DC, TPAD], bf16, kind="ExternalInput")
    # wQ[k, dc, p, cc, j] = weights[dc*128+j, cc*128+p, k]
    w_d = nc.dram_tensor("wP", [K, DC, P, DC, P], bf16, kind="ExternalInput")
    o_d = nc.dram_tensor("outP", [P, DC, NLT, NL], bf16,
                         kind="ExternalOutput")

    with tile.TileContext(nc) as tc:
        with (
            tc.tile_pool(name="wpool", bufs=1) as wpool,
            tc.tile_pool(name="xpool", bufs=1) as xpool,
            tc.tile_pool(name="g0pool", bufs=6) as g0pool,
            tc.tile_pool(name="gpool", bufs=10) as gpool,
            tc.tile_pool(name="apool", bufs=10) as apool,
            tc.tile_pool(name="ppool", bufs=6) as ppool,
            tc.tile_pool(name="psum", bufs=4, space="PSUM") as psum_pool,
        ):
            # xT[p, cc, blk, j] = x[blk*512 + j, cc*128+p]; blocks overlap
            # by K-1 cols so each l-tile is self-contained.
            xT = xpool.tile([P, DC, NLT, BW], bf16, name="xT")
            # w_sb[p, k, dc, cc, j]: lhsT for (cc,k,dc) = w_sb[:,k,dc,cc,:]
            w_sb = wpool.tile([P, K, DC, DC, P], bf16, name="w_sb")

            def ld_x(blk, cc):
                c0 = blk * NL
                nc.sync.dma_start(xT[:, cc, blk, :],
                                  x_d.ap()[:, cc, c0:c0 + BW])

            def ld_w(k, dc, eng):
                eng.dma_start(w_sb[:, k, dc, :, :], w_d.ap()[k, dc])

            # DMA issue order == consumption order; ~128KB descriptors
            # (both sides 1KB-contiguous runs), critical set first. w[k0]
            # rides the scalar DGE queue (idle until the first tanh) in
            # parallel with the x blocks on sync; later weights go on
            # sync between x3 and x4 so they don't steal early bandwidth
            # from the critical x block 0.
            for dc in range(DC):
                ld_w(0, dc, nc.scalar)
            for blk in range(QUAD):
                for cc in range(DC):
                    ld_x(blk, cc)
            for k in range(1, K):
                for dc in range(DC):
                    ld_w(k, dc, nc.sync)
            for blk in range(QUAD, NLT):
                for cc in range(DC):
                    ld_x(blk, cc)

            # PE clock ramps (~1.2 -> 2.4 GHz) after ~3.4us of sustained
            # activity. Dummy matmuls bridge until the critical DMA set
            # lands (~12us: preamble ~7us + issue + ~0.6MB transfer).
            warm = wpool.tile([P, NL], bf16, name="warm")
            nc.gpsimd.memset(warm[:], 1.0)
            warm_ps = psum_pool.tile([P, 2, NL], f32, tag="ps",
                                     name="warm_ps")
            for i in range(NWARM):
                nc.tensor.matmul(warm_ps[:, 0, :], warm[:, :P], warm,
                                 start=True, stop=(i == NWARM - 1))

            for lq in range(NQ):
                q0 = lq * QUAD
                acc = [None] * DC
                if lq == 0:
                    sched = [(0, 0)] + [(k, dc) for k in range(1, K)
                                        for dc in range(DC)]
                else:
                    sched = [(k, dc) for k in range(K) for dc in range(DC)]
                for k, dc in sched:
                    if lq == 0 and k == 0:
                        # Single-l-tile passes: pass i needs only x block
                        # i (+ w[k0]), so real matmuls start ~5us earlier
                        # than a quad-wide sweep needing 4 blocks. d-chunk
                        # pairs share a 2-bank PSUM tile and one tanh.
                        for dc in range(DC):
                            acc[dc] = apool.tile([P, QUAD, NL], bf16,
                                                 tag="acc", name=f"acc0_{dc}")
                        for i in range(QUAD):
                            for dp in range(DC // 2):
                                ps = psum_pool.tile([P, 2, NL], f32,
                                                    tag="ps",
                                                    name=f"ps0_{i}_{dp}")
                                for dh in range(2):
                                    dc = 2 * dp + dh
                                    for cc in range(DC):
                                        nc.tensor.matmul(
                                            ps[:, dh, :],
                                            w_sb[:, 0, dc, cc, :],
                                            xT[:, cc, i, K - 1:BW],
                                            start=(cc == 0),
                                            stop=(cc == DC - 1))
                                g0 = g0pool.tile([P, 2, NL], bf16, tag="g0",
                                                 name=f"g0_{i}_{dp}")
                                nc.scalar.activation(g0, ps, Tanh)
                                for dh in range(2):
                                    dc = 2 * dp + dh
                                    nc.vector.tensor_mul(
                                        acc[dc][:, i, :], g0[:, dh, :],
                                        xT[:, dc, i, 0:NL])
                        continue
                    last_k = (k == K - 1)
                    for dc in (dc,):
                        ps = [psum_pool.tile([P, 2, NL], f32, tag="ps",
                                             name=f"ps_{lq}_{k}_{dc}_{p2}")
                              for p2 in range(2)]
                        if last_k:
                            # pair-wise pipelined epilogue: each 2-bank
                            # PSUM pair closes 8 MMs in, so tanh/mul/add
                            # run under the remaining matmuls and the
                            # post-last-MM tail is one pair deep.
                            g = gpool.tile([P, QUAD, NL], bf16, tag="g",
                                           name=f"gL_{lq}_{dc}")
                            nxt = apool.tile([P, QUAD, NL], bf16, tag="acc",
                                             name=f"accL_{lq}_{dc}")
                            per_lt_store = (lq == NQ - 1 and dc >= DC - 2)
                            # the very last unit runs single-bank tanh per
                            # l-tile so the post-last-matmul chain is one
                            # 512-col tanh/mul/add/store, not a pair's.
                            split_tanh = (lq == NQ - 1 and dc == DC - 1)
                            for p2 in range(2):
                                for ih in range(2):
                                    i = 2 * p2 + ih
                                    for cc in range(DC):
                                        nc.tensor.matmul(
                                            ps[p2][:, ih, :],
                                            w_sb[:, k, dc, cc, :],
                                            xT[:, cc, q0 + i, K - 1:BW],
                                            start=(cc == 0),
                                            stop=(cc == DC - 1))
                                    if split_tanh:
                                        nc.scalar.activation(
                                            g[:, i, :], ps[p2][:, ih, :],
                                            Tanh)
                                if not split_tanh:
                                    nc.scalar.activation(
                                        g[:, 2 * p2:2 * p2 + 2, :], ps[p2],
                                        Tanh)
                                for ih in range(2):
                                    i = 2 * p2 + ih
                                    prod = ppool.tile([P, NL], bf16,
                                                      tag="prodL",
                                                      name=f"prodL_{i}")
                                    nc.vector.tensor_mul(
                                        prod, g[:, i, :],
                                        xT[:, dc, q0 + i, k:k + NL])
                                    nc.vector.tensor_add(
                                        nxt[:, i, :], acc[dc][:, i, :], prod)
                                    if per_lt_store:
                                        nc.sync.dma_start(
                                            o_d.ap()[:, dc, q0 + i, :],
                                            nxt[:, i, :])
                            if not per_lt_store:
                                eng = nc.scalar if lq == NQ - 1 else nc.sync
                                eng.dma_start(
                                    o_d.ap()[:, dc, q0:q0 + QUAD, :],
                                    nxt[:, :, :])
                            acc[dc] = None
                            continue
                        for cc in range(DC):
                            for i in range(QUAD):
                                nc.tensor.matmul(
                                    ps[i // 2][:, i % 2, :],
                                    w_sb[:, k, dc, cc, :],
                                    xT[:, cc, q0 + i, K - 1:BW],
                                    start=(cc == 0), stop=(cc == DC - 1))
                        g = gpool.tile([P, QUAD, NL], bf16, tag="g",
                                       name=f"g_{lq}_{k}_{dc}")
                        for p2 in range(2):
                            nc.scalar.activation(
                                g[:, 2 * p2:2 * p2 + 2, :], ps[p2], Tanh)
                        xu = xT[:, dc, q0:q0 + QUAD, k:k + NL]
                        if acc[dc] is None:
                            a0 = apool.tile([P, QUAD, NL], bf16, tag="acc",
                                            name=f"acc_{lq}_{k}_{dc}")
                            nc.vector.tensor_mul(a0[:, :, :], g[:, :, :], xu)
                            acc[dc] = a0
                        else:
                            prod = ppool.tile([P, QUAD, NL], bf16,
                                              tag="prod",
                                              name=f"prod_{lq}_{k}_{dc}")
                            nc.vector.tensor_mul(prod[:, :, :], g[:, :, :],
                                                 xu)
                            nxt = apool.tile([P, QUAD, NL], bf16, tag="acc",
                                             name=f"accn_{lq}_{k}_{dc}")
                            nc.vector.tensor_add(nxt[:, :, :],
                                                 acc[dc][:, :, :],
                                                 prod[:, :, :])
                            acc[dc] = nxt

    nc.compile()
    return nc


def _prep_inputs(x, weights):
    bf = ml_dtypes.bfloat16
    # wQ[k, dc, p, cc, j] = weights[dc*128+j, cc*128+p, k]
    wP = np.ascontiguousarray(
        weights.reshape(DC, P, DC, P, K).transpose(4, 0, 3, 2, 1)).astype(bf)
    in_maps = []
    for b in range(B):
        xb = np.zeros((P, DC, TPAD), dtype=bf)
        xb[:, :, :T] = x[b].astype(bf).reshape(T, DC, P).transpose(2, 1, 0)
        in_maps.append({"xP": xb, "wP": wP})
    return in_maps


def kernel(x, weights):
    x = np.asarray(x, dtype=np.float32)
    weights = np.asarray(weights, dtype=np.float32)
    assert x.shape == (B, T, C) and weights.shape == (C, C, K)

    from concourse.bass_utils import run_bass_kernel_spmd

    if "nc" not in _cache:
        _cache["nc"] = _build()
    nc = _cache["nc"]

    in_maps = _prep_inputs(x, weights)
    res = run_bass_kernel_spmd(nc, in_maps, list(range(NCORES)))

    out = np.empty((B, L, C), dtype=np.float32)
    for b in range(B):
        op = res.results[b]["outP"].astype(np.float32)  # (P, DC, NLT, NL)
        out[b] = op.transpose(2, 3, 1, 0).reshape(NLT * NL, C)[:L]
    return out


if __name__ == "__main__":
    rng = np.random.default_rng(0)
    x = rng.standard_normal((B, T, C), dtype=np.float32)
    w = (rng.standard_normal((C, C, K), dtype=np.float32)
         / np.sqrt(np.float32(C * K)))
    out = kernel(x, w)
    print("out", out.shape, out.dtype, float(np.abs(out).max()))
